# revision 36
# baseline (speedup 1.0000x reference)
"""Trainium2 Bass kernel for AliasFreeSampling.

Reference op per (b, c) plane X (512x512):
  reflect-pad 32 -> 65-tap separable lowpass -> 2x2 average pool -> Y (256x256)

The whole per-plane operator is linear and separable, so it folds into a
single 512x256 matrix D (pad + conv + pool combined):  Y = D^T @ X @ D.

On the PE array (out = lhsT.T @ rhs, contraction over partitions):
  phase 1: U^T = X^T @ D    via lhsT = X-chunk   [K=i,128][M=w,128],
                                 rhs = D-chunk   [K=i,128][N=j-window]
           -> U^T [w, j] comes out directly, no transposes anywhere.
  phase 2: Y   = U @ D      via lhsT = U^T-chunk [K=w,128][M=j,128],
                                 rhs = D-chunk   [K=w,128][N=c-window]

D is banded (65-tap filter + 2x pool stays local), so a contiguous 128-row
chunk of D only touches a ~96-wide window of the 256 output columns. Each
matmul therefore streams only its chunk's window instead of all 256 columns
(2.9x fewer PE cycles). PSUM accumulation handles the overlapping windows
for free: start=True on the first matmul marks the whole 2 KiB bank
pending-zero, later start=False matmuls add onto zeros (fresh columns) or
partials (overlap columns).

Sharding: pure data parallel - 256 (b,c) planes split as 32 planes on each
of the 8 NeuronCores; D is replicated; no cross-core communication.
"""

import numpy as np

import concourse.bacc as bacc
import concourse.bass as bass
import concourse.mybir as mybir
import concourse.tile as tile
from concourse.bass_utils import run_bass_kernel_spmd

N_CORES = 8
N_PLANES = 32        # planes per core
GROUP = 2            # planes per output-DMA batch
H = W = 512
HO = WO = 256
PAD = 32
TAPS = 65

# matmul dtype mode: "f16"/"bf16" (cast inputs to 16-bit, 1 cycle/row on PE),
# "f32r" (reduced-precision single-pass, broken on HW), "f32" (full, 4x slower)
MM_MODE = "f16"

_MM16 = {"f16": mybir.dt.float16, "bf16": mybir.dt.bfloat16}


def _make_D(k: np.ndarray) -> np.ndarray:
    """Fold reflect-pad(32) + 65-tap conv + 2x avg-pool into one 512x256 map."""
    assert k.shape == (TAPS,)
    D = np.zeros((H, HO), dtype=np.float64)
    t = np.arange(TAPS)
    for j in range(HO):
        for r in (2 * j, 2 * j + 1):
            q = r + t - PAD
            i = np.where(q < 0, -q, np.where(q >= H, 2 * H - 2 - q, q))
            np.add.at(D[:, j], i, 0.5 * k.astype(np.float64))
    return D.astype(np.float32)


def _chunk_windows():
    """Per contiguous 128-row chunk of D, the column support window.

    Computed with all-ones taps: a superset of the true support for any tap
    values, so matmuls streaming just the window are exact. Consecutive
    windows overlap; union covers all 256 columns.
    """
    Dp = _make_D(np.ones(TAPS, dtype=np.float32))
    wins = []
    for c in range(4):
        nz = np.nonzero(np.any(Dp[c * 128:(c + 1) * 128] != 0.0, axis=0))[0]
        j0 = int(nz.min()) & ~1
        j1 = min(HO, (int(nz.max()) + 2) & ~1)
        wins.append((j0, j1))
    for a, b in zip(wins, wins[1:]):
        assert b[0] < a[1], f"windows must overlap for ordering: {wins}"
    return wins


def _emit(tc, y, x, d, n_planes, mode):
    nc = tc.nc
    f32 = mybir.dt.float32
    mm_cast = (lambda ap: ap.bitcast(mybir.dt.float32r)) if mode == "f32r" else (lambda ap: ap)
    WIN = _chunk_windows()

    from contextlib import ExitStack
    with ExitStack() as ctx:
        xpool = ctx.enter_context(tc.tile_pool(name="xin", bufs=3))
        dpool = ctx.enter_context(tc.tile_pool(name="dconst", bufs=1))
        utpool = ctx.enter_context(tc.tile_pool(name="ut", bufs=4))
        ypool = ctx.enter_context(tc.tile_pool(name="yout", bufs=3))
        pspool = ctx.enter_context(tc.tile_pool(name="ps", bufs=1, space="PSUM"))
        # D in natural 128-row chunks: d_sb[p, c, j] = D[128c+p, j]; serves as
        # the moving operand for both phases.
        d_sb = dpool.tile([128, 4, HO], d.dtype)
        dv = d.rearrange("(c p) j -> p c j", p=128)
        # per-chunk loads, c=0 first: the very first matmul only needs chunk 0,
        # so it isn't gated on the full 256 KiB D transfer at cold-DMA rates
        for c in range(4):
            nc.scalar.dma_start(d_sb[:, c], dv[:, c])

        ut_dt = _MM16.get(mode, f32)
        x_dt = _MM16.get(mode, f32)

        # x input split across BOTH HWDGE queues (sync = even planes,
        # scalar = odd): one queue alone is rate-capped ~260GB/s. Each DMA is
        # issued LOOKAHEAD planes before its compute, so on the scalar engine
        # the issue always precedes the casts that could otherwise park it
        # (in-order head-of-line blocking — the failure of the naive split).
        xtiles = {}

        def issue_x(p):
            if p >= n_planes:
                return
            xmm = xpool.tile([128, 4, W], x_dt, tag="x", bufs=16)
            xv = x[p]
            eng = nc.sync if p % 2 == 0 else nc.scalar
            if p == 0:
                # head: fine-grained loads so the first matmul starts
                # after 32 KiB instead of the whole plane
                for c in range(4):
                    for wh in range(2):
                        eng.dma_start(
                            xmm[:, c, wh * 256:(wh + 1) * 256],
                            xv[:, c, wh * 256:(wh + 1) * 256])
            elif p <= 2:
                for c in range(4):
                    eng.dma_start(xmm[:, c], xv[:, c])
            else:
                eng.dma_start(xmm[:], xv)
            xtiles[p] = xmm

        def phase1(p):
            issue_x(p + LOOKAHEAD)
            # partition q holds DRAM rows {128c+q : c in 0..3} (4 KiB
            # contiguous per q); matmul c contracts chunk c's rows against
            # d_sb[:, c]. Two wc's share one PSUM bank [128, 2, 256]
            # (bank-clear on the first matmul covers both halves), one big
            # cast per PAIR — casts have ~250ns fixed cost.
            xmm = xtiles.pop(p)
            ut = utpool.tile([128, 4, HO], ut_dt, tag="ut")
            for pair in range(2):
                ut_ps = pspool.tile([128, 2, HO], f32, tag="utps", bufs=3)
                for sub in range(2):
                    wc = 2 * pair + sub
                    for c in range(4):
                        j0, j1 = WIN[c]
                        nc.tensor.matmul(
                            ut_ps[:, sub, j0:j1],
                            mm_cast(xmm[:, c, wc * 128:(wc + 1) * 128]),
                            mm_cast(d_sb[:, c, j0:j1]),
                            start=(sub == 0 and c == 0),
                            stop=(sub == 1 and c == 3),
                        )
                # casts alternate vector/scalar so no single engine
                # serializes the pipeline (gpsimd can't read PSUM)
                if pair == 0:
                    nc.vector.tensor_copy(ut[:, 0:2, :], ut_ps[:])
                else:
                    nc.scalar.copy(ut[:, 2:4, :], ut_ps[:])
            return ut

        def phase2(q_pl, ut, y_sb):
            # both rr halves accumulate in ONE PSUM bank [128, 2, 256]
            # (start=True only on the first matmul — bank-clear covers the
            # whole 2 KiB bank), one cast.
            pl = q_pl % GROUP
            utv = ut[:].rearrange("q wc (j2 rr) -> q wc j2 rr", rr=2)
            y_ps = pspool.tile([128, 2, WO], f32, tag="yps", bufs=2)
            for rr in range(2):
                for wc in range(4):
                    j0, j1 = WIN[wc]
                    nc.tensor.matmul(
                        y_ps[:, rr, j0:j1],
                        mm_cast(utv[:, wc, :, rr]),
                        mm_cast(d_sb[:, wc, j0:j1]),
                        start=(rr == 0 and wc == 0),
                        stop=(rr == 1 and wc == 3),
                    )
            if q_pl % 2 == 0:
                nc.vector.tensor_copy(y_sb[:, pl, :, :], y_ps[:])
            else:
                nc.scalar.copy(y_sb[:, pl, :, :], y_ps[:])

        LOOKAHEAD = 6
        for p0 in range(LOOKAHEAD):
            issue_x(p0)

        # One-plane skew: tensor stream is ph1(p), ph2(p-1), ph1(p+1), ...
        # so plane p's ut casts complete while plane p+1's phase-1 matmuls
        # stream. Earlier skew attempts regressed because phase-1 starts were
        # gated on x arrival; with the LOOKAHEAD issue that gate is gone.
        uts = {}
        y_sb = None
        for step in range(n_planes + 1):
            if step < n_planes:
                uts[step] = phase1(step)
            if step >= 1:
                q_pl = step - 1
                if q_pl % GROUP == 0:
                    y_sb = ypool.tile([128, GROUP, 2, WO], y.dtype, tag="y")
                phase2(q_pl, uts.pop(q_pl), y_sb)
                if q_pl % GROUP == GROUP - 1:
                    g = q_pl // GROUP
                    nc.scalar.dma_start(
                        y[g * GROUP:(g + 1) * GROUP].rearrange(
                            "pl (q rr) c -> q pl rr c", rr=2),
                        y_sb[:],
                    )


def build_nc(n_planes=N_PLANES, mode=MM_MODE):
    nc = bacc.Bacc("TRN2", target_bir_lowering=False, debug=False)
    f32 = mybir.dt.float32
    d_dt = _MM16.get(mode, f32)
    x_dt = _MM16.get(mode, f32)
    # x pre-permuted on host to [plane, q, c, w] (q = row-within-chunk,
    # c = 128-row chunk): each SBUF partition's 4 KiB loads contiguously.
    x = nc.dram_tensor("x", [n_planes, 128, 4, W], x_dt, kind="ExternalInput").ap()
    d = nc.dram_tensor("d", [H, HO], d_dt, kind="ExternalInput").ap()
    y_dt = _MM16.get(mode, f32)
    y = nc.dram_tensor("y", [n_planes, HO, WO], y_dt, kind="ExternalOutput").ap()
    with tile.TileContext(nc) as tc:
        _emit(tc, y, x, d, n_planes, mode)
    nc.compile()
    return nc


_NC_CACHE = {}


def _get_nc(n_planes=N_PLANES, mode=MM_MODE):
    key = (n_planes, mode)
    if key not in _NC_CACHE:
        _NC_CACHE[key] = build_nc(n_planes, mode)
    return _NC_CACHE[key]


def _d_input(k: np.ndarray, mode: str) -> np.ndarray:
    D = _make_D(k)
    if mode == "f16":
        return D.astype(np.float16)
    if mode == "bf16":
        import ml_dtypes
        return D.astype(ml_dtypes.bfloat16)
    return D


def kernel(x, kernel, **run_kwargs):
    x = np.asarray(x, dtype=np.float32)
    k = np.asarray(kernel, dtype=np.float32)
    B, C = x.shape[0], x.shape[1]
    assert x.shape == (B, C, H, W) and B * C == N_CORES * N_PLANES

    nc = _get_nc()
    d_in = _d_input(k, MM_MODE)
    if MM_MODE == "f16":
        x = x.astype(np.float16)
    elif MM_MODE == "bf16":
        import ml_dtypes
        x = x.astype(ml_dtypes.bfloat16)
    # [plane, h, w] -> [plane, q, c, w]: DRAM layout whose per-partition
    # reads are 4 KiB contiguous (see build_nc)
    xs = x.reshape(N_CORES * N_PLANES, 4, 128, W).transpose(0, 2, 1, 3)
    in_maps = [
        {"x": np.ascontiguousarray(xs[c * N_PLANES:(c + 1) * N_PLANES]), "d": d_in}
        for c in range(N_CORES)
    ]
    res = run_bass_kernel_spmd(nc, in_maps, core_ids=list(range(N_CORES)), **run_kwargs)
    y = np.stack([np.asarray(r["y"], dtype=np.float32) for r in res.results])
    out = y.reshape(B, C, HO, WO)
    if run_kwargs:
        return out, res
    return out


# revision 39
# speedup vs baseline: 1.0748x; 1.0748x over previous
"""Trainium2 Bass kernel for AliasFreeSampling.

Reference op per (b, c) plane X (512x512):
  reflect-pad 32 -> 65-tap separable lowpass -> 2x2 average pool -> Y (256x256)

The whole per-plane operator is linear and separable, so it folds into a
single 512x256 matrix D (pad + conv + pool combined):  Y = D^T @ X @ D.

On the PE array (out = lhsT.T @ rhs, contraction over partitions):
  phase 1: U^T = X^T @ D    via lhsT = X-chunk   [K=i,128][M=w,128],
                                 rhs = D-chunk   [K=i,128][N=j-window]
           -> U^T [w, j] comes out directly, no transposes anywhere.
  phase 2: Y   = U @ D      via lhsT = U^T-chunk [K=w,128][M=j,128],
                                 rhs = D-chunk   [K=w,128][N=c-window]

D is banded (65-tap filter + 2x pool stays local), so a contiguous 128-row
chunk of D only touches a ~96-wide window of the 256 output columns. Each
matmul therefore streams only its chunk's window instead of all 256 columns
(2.9x fewer PE cycles). PSUM accumulation handles the overlapping windows
for free: start=True on the first matmul marks the whole 2 KiB bank
pending-zero, later start=False matmuls add onto zeros (fresh columns) or
partials (overlap columns).

Sharding: pure data parallel - 256 (b,c) planes split as 32 planes on each
of the 8 NeuronCores; D is replicated; no cross-core communication.
"""

import numpy as np

import concourse.bacc as bacc
import concourse.bass as bass
import concourse.mybir as mybir
import concourse.tile as tile
from concourse.bass_utils import run_bass_kernel_spmd

N_CORES = 8
N_PLANES = 32        # planes per core
GROUP = 2            # planes per output-DMA batch
H = W = 512
HO = WO = 256
PAD = 32
TAPS = 65

# matmul dtype mode: "f16"/"bf16" (cast inputs to 16-bit, 1 cycle/row on PE),
# "f32r" (reduced-precision single-pass, broken on HW), "f32" (full, 4x slower)
MM_MODE = "f16"

_MM16 = {"f16": mybir.dt.float16, "bf16": mybir.dt.bfloat16}


def _make_D(k: np.ndarray) -> np.ndarray:
    """Fold reflect-pad(32) + 65-tap conv + 2x avg-pool into one 512x256 map."""
    assert k.shape == (TAPS,)
    D = np.zeros((H, HO), dtype=np.float64)
    t = np.arange(TAPS)
    for j in range(HO):
        for r in (2 * j, 2 * j + 1):
            q = r + t - PAD
            i = np.where(q < 0, -q, np.where(q >= H, 2 * H - 2 - q, q))
            np.add.at(D[:, j], i, 0.5 * k.astype(np.float64))
    return D.astype(np.float32)


def _chunk_windows():
    """Per contiguous 128-row chunk of D, the column support window.

    Computed with all-ones taps: a superset of the true support for any tap
    values, so matmuls streaming just the window are exact. Consecutive
    windows overlap; union covers all 256 columns.
    """
    Dp = _make_D(np.ones(TAPS, dtype=np.float32))
    wins = []
    for c in range(4):
        nz = np.nonzero(np.any(Dp[c * 128:(c + 1) * 128] != 0.0, axis=0))[0]
        j0 = int(nz.min()) & ~1
        j1 = min(HO, (int(nz.max()) + 2) & ~1)
        wins.append((j0, j1))
    for a, b in zip(wins, wins[1:]):
        assert b[0] < a[1], f"windows must overlap for ordering: {wins}"
    return wins


def _emit(tc, y, x, d, n_planes, mode):
    nc = tc.nc
    f32 = mybir.dt.float32
    mm_cast = (lambda ap: ap.bitcast(mybir.dt.float32r)) if mode == "f32r" else (lambda ap: ap)
    WIN = _chunk_windows()

    from contextlib import ExitStack
    with ExitStack() as ctx:
        xpool = ctx.enter_context(tc.tile_pool(name="xin", bufs=3))
        dpool = ctx.enter_context(tc.tile_pool(name="dconst", bufs=1))
        utpool = ctx.enter_context(tc.tile_pool(name="ut", bufs=4))
        ypool = ctx.enter_context(tc.tile_pool(name="yout", bufs=3))
        pspool = ctx.enter_context(tc.tile_pool(name="ps", bufs=1, space="PSUM"))
        # D in natural 128-row chunks: d_sb[p, c, j] = D[128c+p, j]; serves as
        # the moving operand for both phases.
        d_sb = dpool.tile([128, 4, HO], d.dtype)
        dv = d.rearrange("(c p) j -> p c j", p=128)
        # per-chunk loads, c=0 first — and c=0 split so its 20 KiB support
        # window lands first: the very first matmul isn't gated on the full
        # 256 KiB D transfer at cold-DMA rates
        j1_head = WIN[0][1]
        nc.scalar.dma_start(d_sb[:, 0, :j1_head], dv[:, 0, :j1_head])
        nc.scalar.dma_start(d_sb[:, 0, j1_head:], dv[:, 0, j1_head:])
        for c in range(1, 4):
            nc.scalar.dma_start(d_sb[:, c], dv[:, c])

        ut_dt = _MM16.get(mode, f32)
        x_dt = _MM16.get(mode, f32)

        for g in range(n_planes // GROUP):
            y_sb = ypool.tile([128, GROUP, 2, WO], y.dtype, tag="y")
            for pl in range(GROUP):
                p = g * GROUP + pl
                # partition q holds DRAM rows {128c+q : c in 0..3} (1 KiB
                # contiguous per (q, c)); matmul c contracts chunk c's rows
                # against d_sb[:, c].
                xmm = xpool.tile([128, 4, W], x_dt, tag="x", bufs=12)
                xv = x[p]
                if p == 0:
                    # head: fine-grained loads so the first matmul starts
                    # after 32 KiB instead of the whole plane
                    for c in range(4):
                        for wh in range(2):
                            nc.sync.dma_start(
                                xmm[:, c, wh * 256:(wh + 1) * 256],
                                xv[:, c, wh * 256:(wh + 1) * 256])
                elif p == 1:
                    for c in range(4):
                        nc.sync.dma_start(xmm[:, c], xv[:, c])
                else:
                    nc.sync.dma_start(xmm[:], xv)

                # two wc's share one PSUM bank [128, 2, 256] (bank-clear on
                # the first matmul covers both halves), so one big cast per
                # PAIR replaces two — casts have ~250ns fixed cost and the
                # cast engines are the straggler.
                ut = utpool.tile([128, 4, HO], ut_dt, tag="ut")
                for pair in range(2):
                    ut_ps = pspool.tile([128, 2, HO], f32, tag="utps", bufs=3)
                    for sub in range(2):
                        wc = 2 * pair + sub
                        for c in range(4):
                            j0, j1 = WIN[c]
                            nc.tensor.matmul(
                                ut_ps[:, sub, j0:j1],
                                mm_cast(xmm[:, c, wc * 128:(wc + 1) * 128]),
                                mm_cast(d_sb[:, c, j0:j1]),
                                start=(sub == 0 and c == 0),
                                stop=(sub == 1 and c == 3),
                            )
                    # casts split vector/scalar so no single engine serializes
                    # the pipeline (gpsimd can't read PSUM). pair1 finishes
                    # last and gates phase-2's wc2/wc3 weight loads, so it is
                    # split across BOTH engines in parallel to halve that
                    # latency; pair0 has ~0.6µs of slack and stays whole.
                    if pair == 0:
                        nc.vector.tensor_copy(ut[:, 0:2, :], ut_ps[:])
                    else:
                        nc.vector.tensor_copy(ut[:, 2, :], ut_ps[:, 0, :])
                        nc.scalar.copy(ut[:, 3, :], ut_ps[:, 1, :])

                # both rr halves accumulate in ONE PSUM bank [128, 2, 256]
                # (rr0 -> cols 0:256, rr1 -> 256:512): start=True only on the
                # first matmul (bank-clear covers the whole 2 KiB bank), and a
                # single whole-bank cast replaces two half-bank ones
                # (casts have ~250ns fixed cost).
                utv = ut[:].rearrange("q wc (j2 rr) -> q wc j2 rr", rr=2)
                y_ps = pspool.tile([128, 2, WO], f32, tag="yps", bufs=2)
                for rr in range(2):
                    for wc in range(4):
                        j0, j1 = WIN[wc]
                        nc.tensor.matmul(
                            y_ps[:, rr, j0:j1],
                            mm_cast(utv[:, wc, :, rr]),
                            mm_cast(d_sb[:, wc, j0:j1]),
                            start=(rr == 0 and wc == 0),
                            stop=(rr == 1 and wc == 3),
                        )
                if pl % 2 == 0:
                    nc.vector.tensor_copy(y_sb[:, pl, :, :], y_ps[:])
                else:
                    nc.scalar.copy(y_sb[:, pl, :, :], y_ps[:])

            nc.scalar.dma_start(
                y[g * GROUP:(g + 1) * GROUP].rearrange("pl (q rr) c -> q pl rr c", rr=2),
                y_sb[:],
            )


def build_nc(n_planes=N_PLANES, mode=MM_MODE):
    nc = bacc.Bacc("TRN2", target_bir_lowering=False, debug=False)
    f32 = mybir.dt.float32
    d_dt = _MM16.get(mode, f32)
    x_dt = _MM16.get(mode, f32)
    # x pre-permuted on host to [plane, q, c, w] (q = row-within-chunk,
    # c = 128-row chunk): each SBUF partition's 4 KiB loads contiguously.
    x = nc.dram_tensor("x", [n_planes, 128, 4, W], x_dt, kind="ExternalInput").ap()
    d = nc.dram_tensor("d", [H, HO], d_dt, kind="ExternalInput").ap()
    y_dt = _MM16.get(mode, f32)
    y = nc.dram_tensor("y", [n_planes, HO, WO], y_dt, kind="ExternalOutput").ap()
    with tile.TileContext(nc) as tc:
        _emit(tc, y, x, d, n_planes, mode)
    nc.compile()
    return nc


_NC_CACHE = {}


def _get_nc(n_planes=N_PLANES, mode=MM_MODE):
    key = (n_planes, mode)
    if key not in _NC_CACHE:
        _NC_CACHE[key] = build_nc(n_planes, mode)
    return _NC_CACHE[key]


def _d_input(k: np.ndarray, mode: str) -> np.ndarray:
    D = _make_D(k)
    if mode == "f16":
        return D.astype(np.float16)
    if mode == "bf16":
        import ml_dtypes
        return D.astype(ml_dtypes.bfloat16)
    return D


def kernel(x, kernel, **run_kwargs):
    x = np.asarray(x, dtype=np.float32)
    k = np.asarray(kernel, dtype=np.float32)
    B, C = x.shape[0], x.shape[1]
    assert x.shape == (B, C, H, W) and B * C == N_CORES * N_PLANES

    nc = _get_nc()
    d_in = _d_input(k, MM_MODE)
    if MM_MODE == "f16":
        x = x.astype(np.float16)
    elif MM_MODE == "bf16":
        import ml_dtypes
        x = x.astype(ml_dtypes.bfloat16)
    # [plane, h, w] -> [plane, q, c, w]: DRAM layout whose per-partition
    # reads are 4 KiB contiguous (see build_nc)
    xs = x.reshape(N_CORES * N_PLANES, 4, 128, W).transpose(0, 2, 1, 3)
    in_maps = [
        {"x": np.ascontiguousarray(xs[c * N_PLANES:(c + 1) * N_PLANES]), "d": d_in}
        for c in range(N_CORES)
    ]
    res = run_bass_kernel_spmd(nc, in_maps, core_ids=list(range(N_CORES)), **run_kwargs)
    y = np.stack([np.asarray(r["y"], dtype=np.float32) for r in res.results])
    out = y.reshape(B, C, HO, WO)
    if run_kwargs:
        return out, res
    return out
